# revision 22
# baseline (speedup 1.0000x reference)
"""Trainium2 Bass kernel for nn_Denoiser (NNLS-FISTA + BrainNetCNN), 8-core data parallel.

Self-contained: hardcodes shapes/sharding. The harness calls kernel(**inputs) with the
FULL inputs; we shard batch B=64 across 8 NeuronCores (8 samples each), run one SPMD
Bass program, and gather.

Math summary (per sample):
  x0p = (x0_raw - mean) / absmax
  G = Xt^T Xt, C[j,m] = (raw @ X)[j,m]  with X = x0p^T  ([T,M])
  L via power iteration (Rayleigh), step = 1/L
  NNLS-FISTA rewritten so each iteration is pure PE accumulation + 2 elementwise ops:
     U_{k+1} = IG@Wp_k + IG@Wm_{k-1} + D,  IG = I - step*G,
     D = step*C with diagonal = -BIG (replaces the column mask),
     Wp_k = (1+r_k) relu(U_k)   [ACT],  Wm_k = -r_{k+1} relu(U_k)  [DVE]
  fp16 main phase + fp32 polish phase with restarted momentum.
  A = relu(U_final);  BrainNetCNN in fp32.
"""

import numpy as np

B, M, T = 64, 90, 187
NCORES = 8
SH = B // NCORES          # samples per core
NEG = 0.33                # leaky slope
BIG = 1e30
TCH = 128                 # first t-chunk (second is T-TCH=59)
NELEM = float(M * T)

CFG = dict(n16=150, n32=40, npow=20, lsafety=1.02)

WEIGHT_NAMES = [
    "cnn1_w", "cnn1_b", "cnn2_w", "cnn2_b", "cnn3_w", "cnn3_b", "cnn4_w", "cnn4_b",
    "e2n_w", "e2n_b", "n2g_w", "n2g_b", "d1_w", "d1_b", "d2_w", "d2_b", "d3_w", "d3_b",
]


def r_schedule(n):
    """FISTA momentum coefficients r_k = (t_k-1)/t_{k+1}, t_0=1."""
    ts = [1.0]
    for _ in range(n + 2):
        ts.append(0.5 * (1.0 + float(np.sqrt(1.0 + 4.0 * ts[-1] ** 2))))
    return [float(np.float32((ts[k] - 1.0) / ts[k + 1])) for k in range(n + 2)]


def build_program(cfg=None):
    import concourse.bass as bass
    import concourse.mybir as mybir
    import concourse.tile as tile
    import concourse.bass_isa as bass_isa
    from concourse import bacc
    from concourse.masks import make_identity
    from contextlib import ExitStack

    cfg = dict(CFG, **(cfg or {}))
    n16, n32, npow = cfg["n16"], cfg["n32"], cfg["npow"]
    ntot = n16 + n32
    rs1 = r_schedule(ntot)
    rs2 = r_schedule(n32)

    def r_at(k):
        return rs2[k - n16] if k >= n16 else rs1[k]

    dt = mybir.dt
    f32, f16 = dt.float32, dt.float16
    OP = mybir.AluOpType
    AF = mybir.ActivationFunctionType
    AX = mybir.AxisListType
    RO = bass_isa.ReduceOp

    nc = bacc.Bacc("TRN2", target_bir_lowering=False, debug=False, num_devices=NCORES)

    # ---------------- DRAM I/O ----------------
    x0_d = nc.dram_tensor("x0_raw", [SH, M, T], f32, kind="ExternalInput").ap()
    rx_d = nc.dram_tensor("raw_x", [SH, M, T], f32, kind="ExternalInput").ap()
    wd = {}
    wshapes = dict(cnn1_w=[8, M], cnn1_b=[8], cnn2_w=[8, M], cnn2_b=[8],
                   cnn3_w=[16, 8, M], cnn3_b=[16], cnn4_w=[16, 8, M], cnn4_b=[16],
                   e2n_w=[16, M], e2n_b=[1], n2g_w=[64, M], n2g_b=[64],
                   d1_w=[128, 64], d1_b=[128], d2_w=[10, 128], d2_b=[10],
                   d3_w=[2, 10], d3_b=[2])
    for name in WEIGHT_NAMES:
        wd[name] = nc.dram_tensor(name, wshapes[name], f32, kind="ExternalInput").ap()
    x0p_o = nc.dram_tensor("x0p_out", [SH, M, T], f32, kind="ExternalOutput").ap()
    a_o = nc.dram_tensor("a_out", [SH, M, M], f32, kind="ExternalOutput").ap()
    cls_o = nc.dram_tensor("cls_out", [SH, 2], f32, kind="ExternalOutput").ap()

    with ExitStack() as ctx:
        tc = ctx.enter_context(tile.TileContext(nc))
        con = ctx.enter_context(tc.tile_pool(name="con", bufs=1))
        per = ctx.enter_context(tc.tile_pool(name="per", bufs=1))
        tmp = ctx.enter_context(tc.tile_pool(name="tmp", bufs=4))
        small = ctx.enter_context(tc.tile_pool(name="small", bufs=4))
        ldpool = ctx.enter_context(tc.tile_pool(name="ld", bufs=4))
        wp_pool = ctx.enter_context(tc.tile_pool(name="wp", bufs=3))
        wm_pool = ctx.enter_context(tc.tile_pool(name="wm", bufs=3))
        o2pool = ctx.enter_context(tc.tile_pool(name="o2p", bufs=2))
        rowpool = ctx.enter_context(tc.tile_pool(name="rowp", bufs=2))

        def cs(s):
            return slice(s * M, (s + 1) * M)

        # ---------------- constants ----------------
        I128 = con.tile([128, 128], f32)
        ones_col = con.tile([M, 1], f32)
        nc.gpsimd.memset(ones_col[:], 1.0)
        ones_row = con.tile([1, M], f32)
        nc.gpsimd.memset(ones_row[:], 1.0)
        make_identity(nc, I128[:])
        I90 = I128[0:M, 0:M]
        negbig = con.tile([M, M], f32)
        nc.scalar.activation(negbig[:], I90, AF.Copy, scale=-BIG)

        # ---------------- CNN weights (strided DMA views; fp32) -------------
        W1t = con.tile([M, 8], f32)     # [w, o]
        nc.gpsimd.dma_start(W1t[:], wd["cnn1_w"].rearrange("o w -> w o"))
        W2t = con.tile([M, 8], f32)     # [h, o]
        nc.gpsimd.dma_start(W2t[:], wd["cnn2_w"].rearrange("o w -> w o"))
        W3t = con.tile([M, 128], f32)   # [w, (c,o)] : cols c*16+o
        for c in range(8):
            nc.gpsimd.dma_start(W3t[:, c * 16:(c + 1) * 16],
                              wd["cnn3_w"][:, c, :].rearrange("o w -> w o"))
        W4t = con.tile([M, 128], f32)   # [h, (c,o)]
        for c in range(8):
            nc.gpsimd.dma_start(W4t[:, c * 16:(c + 1) * 16],
                              wd["cnn4_w"][:, c, :].rearrange("o w -> w o"))
        Et = con.tile([M, 16], f32)     # [w, c2]
        nc.gpsimd.dma_start(Et[:], wd["e2n_w"].rearrange("c w -> w c"))
        Nt = con.tile([M, 64], f32)     # [h, o]
        nc.gpsimd.dma_start(Nt[:], wd["n2g_w"].rearrange("o h -> h o"))
        d1wt = con.tile([64, 128], f32)
        nc.gpsimd.dma_start(d1wt[:], wd["d1_w"].rearrange("o i -> i o"))
        d2wt = con.tile([128, 10], f32)
        nc.gpsimd.dma_start(d2wt[:], wd["d2_w"].rearrange("o i -> i o"))
        d3wt = con.tile([10, 2], f32)
        nc.gpsimd.dma_start(d3wt[:], wd["d3_w"].rearrange("o i -> i o"))
        # biases
        c1b = con.tile([1, 8], f32)
        nc.gpsimd.dma_start(c1b[:], wd["cnn1_b"].rearrange("(p o) -> p o", p=1))
        c2b = con.tile([1, 8], f32)
        nc.gpsimd.dma_start(c2b[:], wd["cnn2_b"].rearrange("(p o) -> p o", p=1))
        c3b = con.tile([1, 16], f32)
        nc.gpsimd.dma_start(c3b[:], wd["cnn3_b"].rearrange("(p o) -> p o", p=1))
        c4b = con.tile([1, 16], f32)
        nc.gpsimd.dma_start(c4b[:], wd["cnn4_b"].rearrange("(p o) -> p o", p=1))
        e2nb = con.tile([1, 1], f32)
        nc.gpsimd.dma_start(e2nb[:], wd["e2n_b"].rearrange("(p o) -> p o", p=1))
        n2gb = con.tile([64, 1], f32)
        nc.gpsimd.dma_start(n2gb[:], wd["n2g_b"].rearrange("(p o) -> p o", o=1))
        d1b = con.tile([128, 1], f32)
        nc.gpsimd.dma_start(d1b[:], wd["d1_b"].rearrange("(p o) -> p o", o=1))
        d2b = con.tile([10, 1], f32)
        nc.gpsimd.dma_start(d2b[:], wd["d2_b"].rearrange("(p o) -> p o", o=1))
        d3b = con.tile([2, 1], f32)
        nc.gpsimd.dma_start(d3b[:], wd["d3_b"].rearrange("(p o) -> p o", o=1))
        # bias combos as a single [1, 25] row: bsum1(8) | bsum2(16) | e2nb(1);
        # broadcast across partitions later via a ones-row matmul.
        bias_row = con.tile([1, 25], f32)
        nc.vector.tensor_add(bias_row[0:1, 0:8], c1b[:], c2b[:])
        nc.vector.tensor_add(bias_row[0:1, 8:24], c3b[:], c4b[:])
        nc.scalar.activation(bias_row[0:1, 24:25], e2nb[:], AF.Copy)

        # ---------------- load + normalize + transposes + G,C ----------------
        _pSet_cm = tc.tile_pool(name="pSet", bufs=2, space="PSUM")
        pSet = _pSet_cm.__enter__()
        x0p_all = per.tile([M, SH * T], f32)
        Xt_hi = per.tile([TCH, SH * M], f32)
        Xt_lo = per.tile([T - TCH, SH * M], f32)
        Rt_hi = per.tile([TCH, SH * M], f32)
        Rt_lo = per.tile([T - TCH, SH * M], f32)
        G_sb = per.tile([M, SH * M], f32)
        C_sb = per.tile([M, SH * M], f32)

        for s in range(SH):
            xo = ldpool.tile([M, T], f32, tag="xo")
            nc.sync.dma_start(xo[:], x0_d[s])
            ro = ldpool.tile([M, T], f32, tag="ro")
            nc.sync.dma_start(ro[:], rx_d[s])

            red = small.tile([M, 2], f32, tag="red")
            nc.vector.tensor_reduce(red[:, 0:1], xo[:], AX.X, OP.add)
            nc.vector.tensor_reduce(red[:, 1:2], xo[:], AX.X, OP.max,
                                    apply_absolute_value=True)
            sq = pSet.tile([1, 1], f32, tag="sq", bufs=1)
            nc.tensor.matmul(sq[:], lhsT=red[:, 0:1], rhs=ones_col[:],
                             start=True, stop=True)
            tpabs = pSet.tile([1, M], f32, tag="tpabs", bufs=1)
            nc.tensor.matmul(tpabs[:], lhsT=red[:, 1:2], rhs=I90,
                             is_transpose=True, start=True, stop=True)
            nm_rm_row = small.tile([1, 2], f32, tag="nmrow")
            nc.vector.tensor_scalar(out=nm_rm_row[0:1, 0:1], in0=sq[:],
                                    scalar1=-1.0 / NELEM, scalar2=None, op0=OP.mult)
            amax1 = small.tile([1, 1], f32, tag="amax1")
            nc.vector.tensor_reduce(amax1[:], tpabs[:], AX.X, OP.max)
            nc.vector.reciprocal(nm_rm_row[0:1, 1:2], amax1[:])
            nmrm_ps = pSet.tile([M, 2], f32, tag="nmrmps", bufs=1)
            nc.tensor.matmul(nmrm_ps[:], lhsT=ones_row[:], rhs=nm_rm_row[:],
                             start=True, stop=True)
            nm_rm = small.tile([M, 2], f32, tag="nmrm")
            nc.scalar.activation(nm_rm[:], nmrm_ps[:], AF.Copy)
            xs = x0p_all[:, s * T:(s + 1) * T]
            nc.vector.tensor_scalar(out=xs, in0=xo[:], scalar1=nm_rm[:, 0:1],
                                    scalar2=nm_rm[:, 1:2], op0=OP.add, op1=OP.mult)
            nc.sync.dma_start(x0p_o[s], xs)

            # transposes into [t, m] layout (two t-chunks)
            for (src, hi, lo) in ((xs, Xt_hi, Xt_lo), (ro[:], Rt_hi, Rt_lo)):
                t1 = pSet.tile([TCH, M], f32, tag="t1", name="t1")
                nc.tensor.matmul(t1[:], lhsT=src[:, 0:TCH], rhs=I90,
                                 is_transpose=True, start=True, stop=True)
                nc.scalar.activation(hi[:, cs(s)], t1[:], AF.Copy)
                t2 = pSet.tile([T - TCH, M], f32, tag="t2", name="t2", bufs=1)
                nc.tensor.matmul(t2[:], lhsT=src[:, TCH:T], rhs=I90,
                                 is_transpose=True, start=True, stop=True)
                nc.scalar.activation(lo[:, cs(s)], t2[:], AF.Copy)

            gp = pSet.tile([M, M], f32, tag="gp", bufs=1)
            nc.tensor.matmul(gp[:], lhsT=Xt_hi[:, cs(s)], rhs=Xt_hi[:, cs(s)],
                             start=True, stop=False)
            nc.tensor.matmul(gp[:], lhsT=Xt_lo[:, cs(s)], rhs=Xt_lo[:, cs(s)],
                             start=False, stop=True)
            nc.scalar.activation(G_sb[:, cs(s)], gp[:], AF.Copy)
            cp = pSet.tile([M, M], f32, tag="cp", bufs=1)
            nc.tensor.matmul(cp[:], lhsT=Rt_hi[:, cs(s)], rhs=Xt_hi[:, cs(s)],
                             start=True, stop=False)
            nc.tensor.matmul(cp[:], lhsT=Rt_lo[:, cs(s)], rhs=Xt_lo[:, cs(s)],
                             start=False, stop=True)
            nc.scalar.activation(C_sb[:, cs(s)], cp[:], AF.Copy)

        _pSet_cm.__exit__(None, None, None)

        # ---------------- power iteration for L ----------------
        _pPow_cm = tc.tile_pool(name="pPow", bufs=2, space="PSUM")
        pPow = _pPow_cm.__enter__()
        V = per.tile([M, SH], f32)
        nc.gpsimd.memset(V[:], 1.0)
        vcur = V
        for it in range(npow):
            vp = pPow.tile([M, SH], f32, tag="vp", name="vp")
            for s in range(SH):
                nc.tensor.matmul(vp[:, s:s + 1], lhsT=G_sb[:, cs(s)],
                                 rhs=vcur[:, s:s + 1], start=True, stop=True)
            vnew = tmp.tile([M, SH], f32, tag="vnew")
            nc.vector.tensor_scalar(out=vnew[:], in0=vp[:], scalar1=1.0 / 32.0,
                                    scalar2=None, op0=OP.mult)
            vcur = vnew
        vp = pPow.tile([M, SH], f32, tag="vp", name="vp")
        for s in range(SH):
            nc.tensor.matmul(vp[:, s:s + 1], lhsT=G_sb[:, cs(s)],
                             rhs=vcur[:, s:s + 1], start=True, stop=True)
        gv = tmp.tile([M, SH], f32, tag="gv")
        nc.scalar.activation(gv[:], vp[:], AF.Copy)
        qp = pPow.tile([1, 2 * SH], f32, tag="qp", bufs=1)
        for s in range(SH):
            nc.tensor.matmul(qp[0:1, s:s + 1], lhsT=vcur[:, s:s + 1],
                             rhs=gv[:, s:s + 1], start=True, stop=True)
            nc.tensor.matmul(qp[0:1, SH + s:SH + s + 1], lhsT=vcur[:, s:s + 1],
                             rhs=vcur[:, s:s + 1], start=True, stop=True)
        q_sb = small.tile([1, 2 * SH], f32, tag="qsb")
        nc.scalar.activation(q_sb[:], qp[:], AF.Copy)
        rq2 = small.tile([1, SH], f32, tag="rq2")
        nc.vector.reciprocal(rq2[:], q_sb[0:1, SH:2 * SH])
        lam = small.tile([1, SH], f32, tag="lam")
        nc.vector.tensor_mul(lam[:], q_sb[0:1, 0:SH], rq2[:])
        Lrow = small.tile([1, SH], f32, tag="Lrow")
        nc.vector.tensor_scalar(out=Lrow[:], in0=lam[:], scalar1=cfg["lsafety"],
                                scalar2=1e-8, op0=OP.mult, op1=OP.add)
        steprow = small.tile([1, SH], f32, tag="steprow")
        nc.vector.reciprocal(steprow[:], Lrow[:])
        sb_ps = pPow.tile([M, SH], f32, tag="sbps", bufs=1)
        nc.tensor.matmul(sb_ps[:], lhsT=ones_row[:], rhs=steprow[:],
                         start=True, stop=True)
        step_bc = per.tile([M, SH], f32)
        nc.scalar.activation(step_bc[:], sb_ps[:], AF.Copy)
        negstep_bc = per.tile([M, SH], f32)
        nc.vector.tensor_scalar(out=negstep_bc[:], in0=step_bc[:], scalar1=-1.0,
                                scalar2=None, op0=OP.mult)

        _pPow_cm.__exit__(None, None, None)

        # ---------------- IG (f16 + f32) and D ----------------
        IG16 = per.tile([M, SH * M], f16)
        IG32 = per.tile([M, SH * M], f32)
        D = per.tile([M, SH * M], f32)     # [j, m] per sample, diag = -BIG
        for s in range(SH):
            nc.vector.scalar_tensor_tensor(
                out=IG32[:, cs(s)], in0=G_sb[:, cs(s)],
                scalar=negstep_bc[:, s:s + 1], in1=I90, op0=OP.mult, op1=OP.add)
            nc.scalar.activation(IG16[:, cs(s)], IG32[:, cs(s)], AF.Copy)
            nc.vector.scalar_tensor_tensor(
                out=D[:, cs(s)], in0=C_sb[:, cs(s)],
                scalar=step_bc[:, s:s + 1], in1=negbig[:], op0=OP.mult, op1=OP.add)

        # ---------------- FISTA ----------------
        GW = 4 * M  # group width (4 samples)

        def gslice(g):
            return slice(g * GW, (g + 1) * GW)

        _pU_cm = tc.tile_pool(name="pU", bufs=2, space="PSUM")
        pU = _pU_cm.__enter__()
        # U_0 = D (transpose-matmuls)
        U = {}
        for g in (0, 1):
            u = pU.tile([M, GW], f32, tag=f"u{g}")
            for sl in range(4):
                s = g * 4 + sl
                nc.tensor.matmul(u[:, sl * M:(sl + 1) * M], lhsT=D[:, cs(s)],
                                 rhs=I90, is_transpose=True,
                                 start=(sl == 0), stop=(sl == 3))
            U[g] = u

        A_t = per.tile([M, SH * M], f32)   # final relu(U), [m, j]
        Wm_prev = {0: None, 1: None}       # (tile, is_f32)
        for k in range(ntot - 1):
            rk = r_at(k)
            rw = r_at(k + 1)
            wp_f32 = k >= n16
            wm_f32 = (k + 1) >= n16
            for g in (0, 1):
                wp = wp_pool.tile([M, GW], f32 if wp_f32 else f16, tag="wp")
                nc.scalar.activation(wp[:], U[g][:], AF.Relu, scale=1.0 + rk)
                if k < ntot - 2:
                    wm = wm_pool.tile([M, GW], f32 if wm_f32 else f16, tag="wm")
                    nc.vector.tensor_scalar(out=wm[:], in0=U[g][:], scalar1=0.0,
                                            scalar2=-rw, op0=OP.max, op1=OP.mult)
                else:
                    wm = None
                u_new = pU.tile([M, GW], f32, tag=f"u{g}")
                for sl in range(4):
                    s = g * 4 + sl
                    osl = u_new[:, sl * M:(sl + 1) * M]
                    lhs_p = IG32 if wp_f32 else IG16
                    nc.tensor.matmul(osl, lhsT=lhs_p[:, cs(s)],
                                     rhs=wp[:, sl * M:(sl + 1) * M],
                                     start=(sl == 0), stop=False)
                    if Wm_prev[g] is not None:
                        wmp, wmp_f32 = Wm_prev[g]
                        lhs_m = IG32 if wmp_f32 else IG16
                        nc.tensor.matmul(osl, lhsT=lhs_m[:, cs(s)],
                                         rhs=wmp[:, sl * M:(sl + 1) * M],
                                         start=False, stop=False)
                    nc.tensor.matmul(osl, lhsT=D[:, cs(s)], rhs=I90,
                                     is_transpose=True, start=False,
                                     stop=(sl == 3))
                Wm_prev[g] = (wm, wm_f32) if wm is not None else None
                U[g] = u_new
        for g in (0, 1):
            nc.scalar.activation(A_t[:, gslice(g)], U[g][:], AF.Relu)

        _pU_cm.__exit__(None, None, None)

        # ---------------- A output (transpose to [j, m]) ----------------
        _pC1_cm = tc.tile_pool(name="pC1", bufs=2, space="PSUM")
        pC1 = _pC1_cm.__enter__()
        A_sb = per.tile([M, SH * M], f32)   # [h=j, w=m]
        for s in range(SH):
            tp = pC1.tile([M, M], f32, tag="atp", name="atp", bufs=1)
            nc.tensor.matmul(tp[:], lhsT=A_t[:, cs(s)], rhs=I90,
                             is_transpose=True, start=True, stop=True)
            nc.scalar.activation(A_sb[:, cs(s)], tp[:], AF.Copy)
            nc.sync.dma_start(a_o[s], A_sb[:, cs(s)])

        # ---------------- BrainNetCNN (fp32) ----------------
        # layer-1 row/col convs
        aT_ps = pC1.tile([M, SH * 8], f32, tag="aTps", bufs=1)   # [h, (s,o)]
        for s in range(SH):
            nc.tensor.matmul(aT_ps[:, s * 8:(s + 1) * 8], lhsT=A_t[:, cs(s)],
                             rhs=W1t[:], start=(s == 0), stop=(s == SH - 1))
        b_all = per.tile([8, SH * M], f32)               # [o, (s,w)]
        for g in (0, 1):
            b_ps = pC1.tile([8, GW], f32, tag="bps", name="b_ps", bufs=1)
            for sl in range(4):
                s = g * 4 + sl
                nc.tensor.matmul(b_ps[:, sl * M:(sl + 1) * M], lhsT=W2t[:],
                                 rhs=A_sb[:, cs(s)], start=(sl == 0),
                                 stop=(sl == 3))
            nc.scalar.activation(b_all[:, gslice(g)], b_ps[:], AF.Copy)
        # broadcast bias row [1,25] -> [90,25] via ones-row matmul
        bias_ps = pC1.tile([M, 25], f32, tag="biasps", bufs=1)
        nc.tensor.matmul(bias_ps[:], lhsT=ones_row[:], rhs=bias_row[:],
                         start=True, stop=True)
        bias_bc = per.tile([M, 25], f32)
        nc.scalar.activation(bias_bc[:], bias_ps[:], AF.Copy)
        bsum1 = bias_bc[:, 0:8]
        bsum2 = bias_bc[:, 8:24]
        e2nb_bc = bias_bc[:, 24:25]
        # aTb = aT + (c1b+c2b) broadcast over (s, o); also psum->sbuf
        aTb = per.tile([M, SH * 8], f32)
        nc.vector.tensor_add(
            aTb[:].rearrange("p (s o) -> p s o", o=8),
            aT_ps[:].rearrange("p (s o) -> p s o", o=8),
            bsum1[:, None, :].broadcast_to([M, SH, 8]))

        # o1_c = Lrelu(b_bcast + aT_col); both orientations
        o1 = [per.tile([M, SH * M], f32, tag=f"o1_{c}", name=f"o1_{c}") for c in range(8)]
        o1T = [per.tile([M, SH * M], f32, tag=f"o1T_{c}", name=f"o1T_{c}") for c in range(8)]
        aTb_v = aTb[:].rearrange("p (s o) -> p s o", o=8)
        for c in range(8):
            brow = rowpool.tile([1, SH * M], f32, tag="brow", name="brow")
            nc.sync.dma_start(brow[:], b_all[c:c + 1, :])
            for g in (0, 1):
                gsl = gslice(g)
                bcp = pC1.tile([M, GW], f32, tag="bcps", name="bcps")
                nc.tensor.matmul(bcp[:], lhsT=ones_row[:], rhs=brow[0:1, gsl],
                                 start=True, stop=True)
                nc.vector.tensor_add(
                    o1[c][:, gsl].rearrange("p (s w) -> p s w", w=M),
                    bcp[:].rearrange("p (s w) -> p s w", w=M),
                    aTb_v[:, g * 4:(g + 1) * 4, c:c + 1].broadcast_to([M, 4, M]))
            # leaky(x) = max(x, NEG*x) in place (DVE; Pool lacks this op)
            nc.vector.scalar_tensor_tensor(out=o1[c][:], in0=o1[c][:], scalar=NEG,
                                           in1=o1[c][:], op0=OP.mult, op1=OP.max)
            # transpose to o1T
            for g in (0, 1):
                tp = pC1.tile([M, GW], f32, tag="o1tp", name="o1tp")
                for sl in range(4):
                    s = g * 4 + sl
                    nc.tensor.matmul(tp[:, sl * M:(sl + 1) * M],
                                     lhsT=o1[c][:, cs(s)], rhs=I90,
                                     is_transpose=True, start=(sl == 0),
                                     stop=(sl == 3))
                if c % 2 == 0:
                    nc.vector.tensor_copy(o1T[c][:, gslice(g)], tp[:])
                else:
                    nc.scalar.activation(o1T[c][:, gslice(g)], tp[:], AF.Copy)

        _pC1_cm.__exit__(None, None, None)

        # layer-2 convs: a2T[h,(s,o)] and b2T[w,(s,o)], o=16
        _pC2_cm = tc.tile_pool(name="pC2", bufs=2, space="PSUM")
        pC2 = _pC2_cm.__enter__()
        a2T_ps = pC2.tile([M, SH * 16], f32, tag="a2Tps", bufs=1)
        for s in range(SH):
            for c in range(8):
                nc.tensor.matmul(a2T_ps[:, s * 16:(s + 1) * 16],
                                 lhsT=o1T[c][:, cs(s)], rhs=W3t[:, c * 16:(c + 1) * 16],
                                 start=(s == 0 and c == 0),
                                 stop=(s == SH - 1 and c == 7))
        b2T_ps = pC2.tile([M, SH * 16], f32, tag="b2Tps", bufs=1)
        for s in range(SH):
            for c in range(8):
                nc.tensor.matmul(b2T_ps[:, s * 16:(s + 1) * 16],
                                 lhsT=o1[c][:, cs(s)], rhs=W4t[:, c * 16:(c + 1) * 16],
                                 start=(s == 0 and c == 0),
                                 stop=(s == SH - 1 and c == 7))
        a2Tb = per.tile([M, SH * 16], f32)
        nc.vector.tensor_add(
            a2Tb[:].rearrange("p (s o) -> p s o", o=16),
            a2T_ps[:].rearrange("p (s o) -> p s o", o=16),
            bsum2[:, None, :].broadcast_to([M, SH, 16]))
        b2T = per.tile([M, SH * 16], f32)
        nc.scalar.activation(b2T[:], b2T_ps[:], AF.Copy)
        # a2 rows [16, (s,h)]: transpose a2Tb per sample
        a2r = per.tile([16, SH * M], f32)
        for g in (0, 1):
            tp = pC2.tile([16, GW], f32, tag="a2tp", name="a2tp", bufs=1)
            for sl in range(4):
                s = g * 4 + sl
                nc.tensor.matmul(tp[:, sl * M:(sl + 1) * M],
                                 lhsT=a2Tb[:, s * 16:(s + 1) * 16], rhs=I90,
                                 is_transpose=True, start=(sl == 0), stop=(sl == 3))
            nc.scalar.activation(a2r[:, gslice(g)], tp[:], AF.Copy)

        # o2T[w,(s,h)] per c2 = a2_row_bcast + b2T_col (pre-act); leaky folded as
        # NEG*linear + (1-NEG)*relu into two PE contract accumulators.
        e2nA_ps = pC2.tile([M, SH], f32, tag="e2nA", bufs=1)
        e2nB_ps = pC2.tile([M, SH], f32, tag="e2nB", bufs=1)
        b2T_v = b2T[:].rearrange("p (s o) -> p s o", o=16)
        for c2 in range(16):
            a2row = rowpool.tile([1, SH * M], f32, tag="a2row", name="a2row")
            nc.sync.dma_start(a2row[:], a2r[c2:c2 + 1, :])
            o2p = o2pool.tile([M, SH * M], f32, tag="o2p", name="o2p")
            for g in (0, 1):
                gsl = gslice(g)
                bcp = pC2.tile([M, GW], f32, tag="bc2ps", name="bc2ps")
                nc.tensor.matmul(bcp[:], lhsT=ones_row[:], rhs=a2row[0:1, gsl],
                                 start=True, stop=True)
                nc.vector.tensor_add(
                    o2p[:, gsl].rearrange("p (s h) -> p s h", h=M),
                    bcp[:].rearrange("p (s h) -> p s h", h=M),
                    b2T_v[:, g * 4:(g + 1) * 4, c2:c2 + 1].broadcast_to([M, 4, M]))
            o2r = o2pool.tile([M, SH * M], f32, tag="o2r", name="o2r")
            nc.scalar.activation(o2r[:], o2p[:], AF.Relu)
            for s in range(SH):
                nc.tensor.matmul(e2nA_ps[:, s:s + 1], lhsT=o2p[:, cs(s)],
                                 rhs=Et[:, c2:c2 + 1],
                                 start=(c2 == 0 and s == 0),
                                 stop=(c2 == 15 and s == SH - 1),
                                 skip_group_check=True)
                nc.tensor.matmul(e2nB_ps[:, s:s + 1], lhsT=o2r[:, cs(s)],
                                 rhs=Et[:, c2:c2 + 1],
                                 start=(c2 == 0 and s == 0),
                                 stop=(c2 == 15 and s == SH - 1),
                                 skip_group_check=True)
        e2n_pre = small.tile([M, SH], f32, tag="e2npre")
        nc.vector.tensor_scalar(out=e2n_pre[:], in0=e2nA_ps[:], scalar1=NEG,
                                scalar2=None, op0=OP.mult)
        nc.vector.scalar_tensor_tensor(out=e2n_pre[:], in0=e2nB_ps[:],
                                       scalar=1.0 - NEG, in1=e2n_pre[:],
                                       op0=OP.mult, op1=OP.add)
        e2n_sb = small.tile([M, SH], f32, tag="e2n")
        nc.scalar.activation(e2n_sb[:], e2n_pre[:], AF.Identity, bias=e2nb_bc[:])
        nc.vector.scalar_tensor_tensor(out=e2n_sb[:], in0=e2n_sb[:], scalar=NEG,
                                       in1=e2n_sb[:], op0=OP.mult, op1=OP.max)

        # dense head
        def dense(lhsT, rhs, bias, pdim, name):
            hp = pC2.tile([pdim, SH], f32, tag="hp", name=f"hp_{name}", bufs=1)
            nc.tensor.matmul(hp[:], lhsT=lhsT, rhs=rhs, start=True, stop=True)
            hs = small.tile([pdim, SH], f32, tag=f"hs_{name}", name=f"hs_{name}")
            nc.scalar.activation(hs[:], hp[:], AF.Identity, bias=bias)
            nc.vector.scalar_tensor_tensor(out=hs[:], in0=hs[:], scalar=NEG,
                                           in1=hs[:], op0=OP.mult, op1=OP.max)
            return hs

        h1 = dense(Nt[:], e2n_sb[:], n2gb[:], 64, "n2g")
        h2 = dense(d1wt[:], h1[:], d1b[:], 128, "d1")
        h3 = dense(d2wt[:], h2[:], d2b[:], 10, "d2")
        cls_sb = dense(d3wt[:], h3[:], d3b[:], 2, "d3")
        nc.sync.dma_start(cls_o.rearrange("s o -> o s"), cls_sb[:])
        _pC2_cm.__exit__(None, None, None)

    nc.compile()
    return nc


_prog_cache = {}


def _get_program(cfg_key=None):
    key = tuple(sorted((cfg_key or CFG).items()))
    if key not in _prog_cache:
        _prog_cache[key] = build_program(dict(key))
    return _prog_cache[key]


def run(inputs, trace=False, cfg=None):
    from concourse.bass_utils import run_bass_kernel_spmd

    nc = _get_program(cfg)
    core_ids = list(range(NCORES))
    in_maps = []
    for c in core_ids:
        m = {
            "x0_raw": np.ascontiguousarray(inputs["x0_raw"][c * SH:(c + 1) * SH]),
            "raw_x": np.ascontiguousarray(inputs["raw_x"][c * SH:(c + 1) * SH]),
        }
        for name in WEIGHT_NAMES:
            m[name] = np.ascontiguousarray(inputs[name], dtype=np.float32)
        in_maps.append(m)
    bkr = run_bass_kernel_spmd(nc, in_maps, core_ids, trace=trace)
    res = bkr.results
    x0p = np.concatenate([res[c]["x0p_out"] for c in core_ids], axis=0)
    A = np.concatenate([res[c]["a_out"] for c in core_ids], axis=0)
    cls = np.concatenate([res[c]["cls_out"] for c in core_ids], axis=0)
    outs = (x0p[:, None, :, :].astype(np.float32),
            A.astype(np.float32), cls.astype(np.float32))
    return (outs, bkr) if trace else outs


def kernel(**inputs):
    return run(inputs, trace=False)
